# revision 34
# baseline (speedup 1.0000x reference)
"""Trainium2 Bass kernel for the FCNN color-counter valuation function.

Computes out[i] = a[i, int(z[i, attr_index])] * 0.999 for i in [0, B).

Strategy: pure data parallel over 8 NeuronCores (batch sharded). Per core,
rows are laid out partition-major ([128 partitions, J rows each]) so every
DMA is a large contiguous-per-partition transfer.

The gather runs per color category on the vector engine:
    prod[:, :, c] = (z[:, :, attr] == c) * a[:, :, c]   # 10x scalar_tensor_tensor
    red           = reduce_add(prod, axis=C)            # exact: one nonzero/row
    out           = red * 0.999                         # on ACT engine
This costs ~20.5 DVE cycles/row vs ~30 for the broadcast-mask scheme (the
mask build and the fused multiply both run at 1x because of the stride-0
broadcast operand; the per-color form keeps every op at 1x but touches each
a element only twice instead of three times). Result is bit-exact vs the
f32 reference.

Loads ride the SP HWDGE ring; the 0.999 scale and per-tile output stores ride
the ACT engine/ring so they overlap the loads. Tile sizes taper at both ends:
small head tiles fill the pipeline quickly, small tail tiles shrink the
compute+store tail after the last load completes.
"""

import numpy as np

import concourse.bacc as bacc
import concourse.mybir as mybir
import concourse.tile as tile
from concourse import bass_utils

B = 2097152  # total batch rows
D = 16       # z feature width
C = 10       # color-counter categories
NCORES = 8
R = B // NCORES   # rows per core = 262144
P = 128           # SBUF partitions
J = R // P        # rows per partition = 2048

_cache: dict[tuple, "bacc.Bacc"] = {}

# Tunables (overridable for A/B benchmarking).
DEFAULTS = dict(
    tile_sizes=(128,) * 15 + (64, 32, 32),
    io_bufs=8,
    scheme="maskbf",   # "maskbf" | "maskgp" | "percolor" | "mask"
    scale_engine="scalar",  # engine for the *0.999 (percolor): "scalar"|"vector"
    store_engine="scalar",  # engine ring for output stores
    gp_colors=0,  # how many of the C per-color ops run on GPSIMD (rest DVE)
)


def _build(attr_index: int, tile_sizes, io_bufs=3, scheme="percolor",
           scale_engine="scalar", store_engine="scalar",
           gp_colors=0) -> "bacc.Bacc":
    tile_sizes = tuple(tile_sizes)
    assert sum(tile_sizes) == J

    nc = bacc.Bacc("TRN2", target_bir_lowering=False, debug=False)

    z_d = nc.dram_tensor("z", [R, D], mybir.dt.float32, kind="ExternalInput")
    a_d = nc.dram_tensor("a", [R, C], mybir.dt.float32, kind="ExternalInput")
    o_d = nc.dram_tensor("out", [R], mybir.dt.float32, kind="ExternalOutput")

    # Partition-major row layout: local row r -> (partition r // J, slot r % J).
    z_t = z_d.ap().rearrange("(p j) d -> p j d", p=P)
    a_t = a_d.ap().rearrange("(p j) c -> p j c", p=P)
    o_t = o_d.ap().rearrange("(p j) -> p j", p=P)

    qmax = max(tile_sizes)

    with tile.TileContext(nc) as tc:
        with (
            tc.tile_pool(name="const", bufs=1) as constp,
            tc.tile_pool(name="zp", bufs=8) as zp,
            tc.tile_pool(name="io", bufs=io_bufs) as iop,
            tc.tile_pool(name="work",
                         bufs=4 if scheme == "maskbf"
                         else 2 if scheme == "maskgp" else 1) as workp,
            tc.tile_pool(name="osb",
                         bufs=6 if scheme == "maskbf" else 2) as outp,
        ):
            st_eng = nc.scalar if store_engine == "scalar" else nc.sync

            iota_f = None
            if scheme in ("mask", "maskgp", "maskbf"):
                # Build the 0..C-1 ramp with DVE memsets: gpsimd.iota would
                # trigger a ~6us Pool library IRAM load inside the startup
                # barrier, delaying the first DMA by that much.
                iota_f = constp.tile([P, C], mybir.dt.float32)
                for c in range(C):
                    nc.vector.memset(iota_f[:, c : c + 1], float(c))

            T = len(tile_sizes)
            starts = [sum(tile_sizes[:t]) for t in range(T)]

            if scheme == "maskbf":
                # All-DVE bf16 pipeline:
                #   mask = (z8 == iota)        TT, 1x (broadcast), bf16 out
                #   prod = mask * (a * 0.999)  TT, 2x (all-dense bf16)
                #   red  = sum_C prod          reduce, 1x, f32 out
                # ACT pre-scales a by 0.999 while casting to bf16, and
                # issues the stores. Software-pipelined by one tile so
                # neither in-order engine stalls on the other: DVE runs
                # mask(t) before product(t-1); ACT runs cast(t) before
                # store(t-1). bf16 worst-case rel err ~2^-8, well under
                # the 2e-2 gate.
                T = len(tile_sizes)
                pend = None  # (mask, sl, q, a_f32) awaiting product+reduce

                def flush_bf(pend, abf_of):
                    mask_p, sl_p, q_p, a_f32 = pend
                    if abf_of is None:
                        # Tail tiles: fused f32 product (mask*0.999)*a
                        # straight from the f32 a tile — no ACT-cast link
                        # in the chain, so the post-load tail stays short.
                        nc.vector.scalar_tensor_tensor(
                            out=mask_p, in0=mask_p, scalar=0.999,
                            in1=a_f32,
                            op0=mybir.AluOpType.mult,
                            op1=mybir.AluOpType.mult,
                        )
                    else:
                        nc.vector.tensor_tensor(
                            out=mask_p, in0=mask_p, in1=abf_of,
                            op=mybir.AluOpType.mult,
                        )
                    red = outp.tile([P, q_p], mybir.dt.float32, tag="red",
                                    padded_shape=[P, qmax])
                    nc.vector.tensor_reduce(
                        out=red, in_=mask_p,
                        axis=mybir.AxisListType.X, op=mybir.AluOpType.add,
                    )
                    # Stores ride SWDGE via the otherwise-idle GPSIMD
                    # sequencer: its sem wait on reduce(t) can't block the
                    # load ring (SP) or the cast stream (ACT).
                    nc.gpsimd.dma_start(out=o_t[:, sl_p], in_=red)

                abfs = {}
                for t, q in enumerate(tile_sizes):
                    sl = slice(starts[t], starts[t] + q)
                    z_tile = zp.tile([P, q, D], mybir.dt.float32, tag="zt",
                                     padded_shape=[P, qmax, D])
                    nc.sync.dma_start(out=z_tile, in_=z_t[:, sl, :])
                    a_tile = iop.tile([P, q, C], mybir.dt.float32, tag="at",
                                      padded_shape=[P, qmax, C])
                    nc.sync.dma_start(out=a_tile, in_=a_t[:, sl, :])

                    if t < T - 2:
                        a_bf = workp.tile([P, q, C], mybir.dt.bfloat16,
                                          tag="abf",
                                          padded_shape=[P, qmax, C])
                        nc.scalar.mul(out=a_bf, in_=a_tile, mul=0.999)
                        abfs[t] = a_bf
                    else:
                        abfs[t] = None

                    mask = workp.tile([P, q, C], mybir.dt.bfloat16,
                                      tag="mask", padded_shape=[P, qmax, C])
                    z_b = z_tile[:, :, attr_index : attr_index + 1]
                    z_b = z_b.broadcast_to([P, q, C])
                    i_b = iota_f.unsqueeze(1).broadcast_to([P, q, C])
                    nc.vector.tensor_tensor(
                        out=mask, in0=z_b, in1=i_b,
                        op=mybir.AluOpType.is_equal,
                    )
                    if pend is not None:
                        flush_bf(pend, abfs[t - 1])
                    pend = (mask, sl, q, a_tile)
                flush_bf(pend, abfs[T - 1])

            elif scheme == "maskgp":
                # DVE builds the one-hot mask; GPSIMD multiplies by a in
                # place (dense TT — its only walrus-supported form); DVE
                # reduces; ACT scales and stores. Splitting the three
                # passes across engines keeps each under the HBM-bound
                # load time. The reduce for tile t-1 is issued after the
                # mask for tile t so DVE keeps working while GPSIMD
                # multiplies tile t (prod is double-buffered).
                pending = None  # (prod, sl, q) awaiting reduce+store

                def flush(pending):
                    prod_p, sl_p, q_p = pending
                    red = outp.tile([P, q_p], mybir.dt.float32, tag="red",
                                    padded_shape=[P, qmax])
                    nc.vector.tensor_reduce(
                        out=red, in_=prod_p,
                        axis=mybir.AxisListType.X, op=mybir.AluOpType.add,
                    )
                    sc = outp.tile([P, q_p], mybir.dt.float32, tag="sc",
                                   padded_shape=[P, qmax])
                    nc.scalar.mul(out=sc, in_=red, mul=0.999)
                    st_eng.dma_start(out=o_t[:, sl_p], in_=sc)

                for t, q in enumerate(tile_sizes):
                    sl = slice(starts[t], starts[t] + q)
                    z_tile = iop.tile([P, q, D], mybir.dt.float32, tag="zt",
                                      padded_shape=[P, qmax, D])
                    nc.sync.dma_start(out=z_tile, in_=z_t[:, sl, :])
                    a_tile = iop.tile([P, q, C], mybir.dt.float32, tag="at",
                                      padded_shape=[P, qmax, C])
                    nc.sync.dma_start(out=a_tile, in_=a_t[:, sl, :])

                    prod = workp.tile([P, q, C], mybir.dt.float32,
                                      tag="prod", padded_shape=[P, qmax, C])
                    z_b = z_tile[:, :, attr_index : attr_index + 1]
                    z_b = z_b.broadcast_to([P, q, C])
                    i_b = iota_f.unsqueeze(1).broadcast_to([P, q, C])
                    nc.vector.tensor_tensor(
                        out=prod, in0=z_b, in1=i_b,
                        op=mybir.AluOpType.is_equal,
                    )
                    nc.gpsimd.tensor_tensor(
                        out=prod, in0=prod, in1=a_tile,
                        op=mybir.AluOpType.mult,
                    )
                    if pending is not None:
                        flush(pending)
                    pending = (prod, sl, q)
                flush(pending)

            for t, q in (enumerate(tile_sizes)
                         if scheme not in ("maskgp", "maskbf") else ()):
                sl = slice(starts[t], starts[t] + q)

                z_tile = iop.tile([P, q, D], mybir.dt.float32, tag="zt",
                                  padded_shape=[P, qmax, D])
                nc.sync.dma_start(out=z_tile, in_=z_t[:, sl, :])
                a_tile = iop.tile([P, q, C], mybir.dt.float32, tag="at",
                                  padded_shape=[P, qmax, C])
                nc.sync.dma_start(out=a_tile, in_=a_t[:, sl, :])

                prod = workp.tile([P, q, C], mybir.dt.float32, tag="prod",
                                  padded_shape=[P, qmax, C])
                red = outp.tile([P, q], mybir.dt.float32, tag="red",
                                padded_shape=[P, qmax])

                if scheme == "percolor":
                    z8 = z_tile[:, :, attr_index : attr_index + 1]
                    for c in range(C):
                        if c >= C - gp_colors:
                            # Pool rejects the fused stt; split into ts + tt.
                            e_c = workp.tile([P, q], mybir.dt.float32,
                                             tag=f"e{c % 2}",
                                             padded_shape=[P, qmax])
                            nc.gpsimd.tensor_scalar(
                                e_c.unsqueeze(2), z8, float(c), None,
                                mybir.AluOpType.is_equal,
                            )
                            nc.gpsimd.tensor_tensor(
                                out=prod[:, :, c : c + 1],
                                in0=e_c.unsqueeze(2),
                                in1=a_tile[:, :, c : c + 1],
                                op=mybir.AluOpType.mult,
                            )
                        else:
                            nc.vector.scalar_tensor_tensor(
                                out=prod[:, :, c : c + 1],
                                in0=z8,
                                scalar=float(c),
                                in1=a_tile[:, :, c : c + 1],
                                op0=mybir.AluOpType.is_equal,
                                op1=mybir.AluOpType.mult,
                            )
                    nc.vector.tensor_reduce(
                        out=red,
                        in_=prod,
                        axis=mybir.AxisListType.X,
                        op=mybir.AluOpType.add,
                    )
                else:
                    z_b = z_tile[:, :, attr_index : attr_index + 1].broadcast_to(
                        [P, q, C]
                    )
                    i_b = iota_f.unsqueeze(1).broadcast_to([P, q, C])
                    nc.vector.tensor_tensor(
                        out=prod, in0=z_b, in1=i_b,
                        op=mybir.AluOpType.is_equal,
                    )
                    nc.vector.scalar_tensor_tensor(
                        out=prod, in0=prod, scalar=1.0, in1=a_tile,
                        op0=mybir.AluOpType.mult, op1=mybir.AluOpType.mult,
                    )
                    nc.vector.tensor_reduce(
                        out=red, in_=prod,
                        axis=mybir.AxisListType.X, op=mybir.AluOpType.add,
                    )

                sc = outp.tile([P, q], mybir.dt.float32, tag="sc",
                               padded_shape=[P, qmax])
                if scale_engine == "scalar":
                    nc.scalar.mul(out=sc, in_=red, mul=0.999)
                else:
                    nc.vector.tensor_scalar_mul(sc, red, 0.999)
                st_eng.dma_start(out=o_t[:, sl], in_=sc)

    nc.compile()
    return nc


def get_nc(attr_index: int = 8, **opts) -> "bacc.Bacc":
    cfg = dict(DEFAULTS)
    cfg.update(opts)
    cfg["tile_sizes"] = tuple(cfg["tile_sizes"])
    key = (int(attr_index), tuple(sorted(cfg.items())))
    if key not in _cache:
        _cache[key] = _build(int(attr_index), **cfg)
    return _cache[key]


def run(z, a, attr_index=8, trace: bool = False, **opts):
    """Run on all 8 cores; returns (full_output, BassKernelResults)."""
    nc = get_nc(attr_index, **opts)
    z = np.ascontiguousarray(np.asarray(z, dtype=np.float32))
    a = np.ascontiguousarray(np.asarray(a, dtype=np.float32))
    assert z.shape == (B, D) and a.shape == (B, C), (z.shape, a.shape)
    in_maps = [
        {"z": z[i * R : (i + 1) * R], "a": a[i * R : (i + 1) * R]}
        for i in range(NCORES)
    ]
    res = bass_utils.run_bass_kernel_spmd(
        nc, in_maps, core_ids=list(range(NCORES)), trace=trace
    )
    out = np.concatenate([r["out"].reshape(R) for r in res.results])
    return out, res


def kernel(z, a, attr_index=8, **_unused):
    out, _ = run(z, a, attr_index)
    return out
